# revision 16
# baseline (speedup 1.0000x reference)
"""DeepSeekMOE grouped-GEMM kernel for 8 Trainium2 NeuronCores.

Expert-parallel: core g handles expert group g.
Per core:  h = x @ w_up_gate ; act = silu(gate)*up ; out = act @ w_down
with x:[1536,2048], w_up_gate:[2048,2816], w_down:[1408,2048] (fp32).

Dataflow (transpose-free on device):
  - host supplies xT = x.T  ([2048,1536]) so both GEMM operands have the
    contraction dim on partitions.
  - GEMM1 computes hT tiles ([n_chunk 128, m 512]) = w1_colblock.T @ xT,
    so SwiGLU output actT lands directly in [E, M] layout — exactly the
    stationary-operand layout GEMM2 needs. out = actT.T @ w_down comes out
    in natural [M, H] orientation.

Matmul inputs are bf16 (host-converted); accumulation stays fp32 in PSUM
and the output is fp32. Set IN_DT = F32R for a TF32-class variant (~2.6e-4
rel err instead of ~5e-3) at ~10% more runtime (doubled DMA traffic).

DMA queues: weights stream on the sync-engine HWDGE queue; xT, w_down and
output tiles on the scalar-engine HWDGE queue (queues execute in emission
order, so the first pair's weights are not stuck behind the xT load).
"""

import sys
import numpy as np

if "/opt/trn_rl_repo" not in sys.path:
    sys.path.insert(0, "/opt/trn_rl_repo")

import ml_dtypes
import concourse.bass as bass
import concourse.bacc as bacc
import concourse.mybir as mybir
import concourse.tile as tile
from concourse.bass_utils import run_bass_kernel_spmd

P = 128
M = 1536          # tokens per expert group
K = 2048          # hidden
N2 = 2816         # 2 * expert_dim (gate | up)
E = 1408          # expert_dim
H = 2048          # hidden (output)

KC = K // P       # 16 contraction chunks, GEMM1
EC = E // P       # 11 contraction chunks, GEMM2 / n-pairs
MT = 512          # m free-dim tile
NMT = M // MT     # 3 m-tiles
MC = M // P       # 12 output m-chunks, GEMM2
HT = 512          # h free-dim tile
NHT = H // HT     # 4 h-tiles

F32 = mybir.dt.float32
F32R = mybir.dt.float32r
BF16 = mybir.dt.bfloat16

IN_DT = BF16      # matmul input dtype (BF16 or F32R)

_cache = {}


def _np_in_dtype():
    return ml_dtypes.bfloat16 if IN_DT == BF16 else np.float32


def _build_nc():
    nc = bacc.Bacc("TRN2", target_bir_lowering=False)

    xT = nc.declare_dram_parameter("xT", [K, M], IN_DT, isOutput=False)
    # w1 arrives host-packed as [pair, gate/up, p, k, c] so each column
    # block is a fully-contiguous DMA (4 KB/partition packets, full rate)
    w1 = nc.declare_dram_parameter("w1", [EC, 2, P, KC, P], IN_DT,
                                   isOutput=False)
    w2 = nc.declare_dram_parameter("w2", [E, H], IN_DT, isOutput=False)
    out = nc.declare_dram_parameter("out", [M, H], F32, isOutput=True)

    with tile.TileContext(nc) as tc:
        with tc.tile_pool(name="act", bufs=1) as act_pool, \
             tc.tile_pool(name="w2p", bufs=1) as w2_pool, \
             tc.tile_pool(name="xt", bufs=1) as xt_pool, \
             tc.tile_pool(name="w1p", bufs=4) as w1_pool, \
             tc.tile_pool(name="silu", bufs=4) as silu_pool, \
             tc.tile_pool(name="ost", bufs=4) as out_pool, \
             tc.tile_pool(name="ps", bufs=8, space="PSUM") as ps_pool:

            # actT: [E, M], resident through both phases
            act_t = [act_pool.tile([P, M], IN_DT, name=f"act{e}", tag=f"act{e}")
                     for e in range(EC)]

            wgs, wus = {}, {}

            def load_pair_weights(i):
                wg = w1_pool.tile([P, KC, P], IN_DT, name=f"wg{i}", tag="wg")
                wu = w1_pool.tile([P, KC, P], IN_DT, name=f"wu{i}", tag="wu")
                nc.sync.dma_start(out=wg, in_=w1[i, 0])
                nc.sync.dma_start(out=wu, in_=w1[i, 1])
                wgs[i], wus[i] = wg, wu

            # ---------------- Phase 1: GEMM1 + SwiGLU ----------------
            # weights stream on the sync queue; pairs 0-2 first
            load_pair_weights(0)
            load_pair_weights(1)
            load_pair_weights(2)

            # xT streams on the scalar queue as [128, 512] m-slices in
            # t-major order — the order the startup MM block consumes them.
            xts = [[None] * KC for _ in range(NMT)]
            for t in range(NMT):
                for k in range(KC):
                    xt = xt_pool.tile([P, MT], IN_DT, name=f"xt{t}_{k}",
                                      tag=f"xt{t}_{k}")
                    nc.scalar.dma_start(
                        out=xt,
                        in_=xT[k * P:(k + 1) * P, t * MT:(t + 1) * MT])
                    xts[t][k] = xt

            # w_down preload tiles; DMAs are emitted later on the sync queue
            # (after all w1 loads) so they don't compete with xT streaming
            w2ts = [w2_pool.tile([P, H], IN_DT, name=f"w2t{e}", tag=f"w2t{e}")
                    for e in range(EC)]

            def psum_pair(i, t):
                g = ps_pool.tile([P, MT], F32, name=f"psg{i}_{t}", tag="ps")
                u = ps_pool.tile([P, MT], F32, name=f"psu{i}_{t}", tag="ps")
                return g, u

            def swiglu(i, t, g, u):
                tmp = silu_pool.tile([P, MT], F32, name=f"silu{i}_{t}",
                                     tag="silu")
                nc.scalar.activation(
                    out=tmp, in_=g, func=mybir.ActivationFunctionType.Silu)
                nc.vector.tensor_mul(
                    out=act_t[i][:, t * MT:(t + 1) * MT], in0=tmp, in1=u)

            # pairs 0..10 sequential, t-outer/k-inner — pair 0's demand
            # order (t-major, per-chunk) matches the DMA supply order
            for i in range(0, 2):
                wg, wu = wgs.pop(i), wus.pop(i)
                for t in range(NMT):
                    g, u = psum_pair(i, t)
                    for k in range(KC):
                        st, sp = (k == 0), (k == KC - 1)
                        xk = xts[t][k]
                        nc.tensor.matmul(g, wg[:, k, :], xk, start=st, stop=sp)
                        nc.tensor.matmul(u, wu[:, k, :], xk, start=st, stop=sp)
                    swiglu(i, t, g, u)

            # steady state: pairs 2..10
            for i in range(2, EC):
                if i + 1 < EC:
                    load_pair_weights(i + 1)
                else:
                    # all w1 queued — append w_down preloads to the sync queue
                    for e in range(EC):
                        nc.sync.dma_start(out=w2ts[e],
                                          in_=w2[e * P:(e + 1) * P, :])
                wg, wu = wgs.pop(i), wus.pop(i)
                for t in range(NMT):
                    g, u = psum_pair(i, t)
                    for k in range(KC):
                        st, sp = (k == 0), (k == KC - 1)
                        xk = xts[t][k]
                        nc.tensor.matmul(g, wg[:, k, :], xk, start=st, stop=sp)
                        nc.tensor.matmul(u, wu[:, k, :], xk, start=st, stop=sp)
                    swiglu(i, t, g, u)

            # ---------------- Phase 2: GEMM2 ----------------
            for mc in range(MC):
                ps_o = [ps_pool.tile([P, HT], F32, name=f"pso{mc}_{h}",
                                     tag="ps") for h in range(NHT)]
                for e in range(EC):
                    for h in range(NHT):
                        nc.tensor.matmul(
                            ps_o[h],
                            act_t[e][:, mc * P:(mc + 1) * P],
                            w2ts[e][:, h * HT:(h + 1) * HT],
                            start=(e == 0), stop=(e == EC - 1))
                for h in range(NHT):
                    ot = out_pool.tile([P, HT], F32, name=f"ot{mc}_{h}",
                                       tag="ot")
                    nc.vector.tensor_copy(out=ot, in_=ps_o[h])
                    nc.scalar.dma_start(
                        out=out[mc * P:(mc + 1) * P, h * HT:(h + 1) * HT],
                        in_=ot)

    nc.compile()
    return nc


def _pack_w1(w1g):
    # [K, 2*E] -> [pair, gate/up, p, k, c]
    a = w1g.reshape(KC, P, 2, EC, P).transpose(3, 2, 1, 0, 4)
    return np.ascontiguousarray(a)


def make_in_maps(x, w_up_gate, w_down):
    dt = _np_in_dtype()
    in_maps = []
    for g in range(x.shape[0]):
        in_maps.append({
            "xT": np.ascontiguousarray(x[g].T).astype(dt),
            "w1": _pack_w1(w_up_gate[g].astype(dt)),
            "w2": np.ascontiguousarray(w_down[g]).astype(dt),
        })
    return in_maps


def kernel(x, w_up_gate, w_down):
    G = x.shape[0]
    if "nc" not in _cache:
        _cache["nc"] = _build_nc()
    nc = _cache["nc"]
    res = run_bass_kernel_spmd(nc, make_in_maps(x, w_up_gate, w_down),
                               list(range(G)))
    return np.stack([res.results[g]["out"] for g in range(G)], axis=0)


# revision 18
# speedup vs baseline: 1.0196x; 1.0196x over previous
"""DeepSeekMOE grouped-GEMM kernel for 8 Trainium2 NeuronCores.

Expert-parallel: core g handles expert group g.
Per core:  h = x @ w_up_gate ; act = silu(gate)*up ; out = act @ w_down
with x:[1536,2048], w_up_gate:[2048,2816], w_down:[1408,2048] (fp32).

Dataflow (transpose-free on device):
  - host supplies xT = x.T  ([2048,1536]) so both GEMM operands have the
    contraction dim on partitions.
  - GEMM1 computes hT tiles ([n_chunk 128, m 512]) = w1_colblock.T @ xT,
    so SwiGLU output actT lands directly in [E, M] layout — exactly the
    stationary-operand layout GEMM2 needs. out = actT.T @ w_down comes out
    in natural [M, H] orientation.

Matmul inputs are bf16 (host-converted); accumulation stays fp32 in PSUM
and the output is fp32. Set IN_DT = F32R for a TF32-class variant (~2.6e-4
rel err instead of ~5e-3) at ~10% more runtime (doubled DMA traffic).

DMA queues: weights stream on the sync-engine HWDGE queue; xT, w_down and
output tiles on the scalar-engine HWDGE queue (queues execute in emission
order, so the first pair's weights are not stuck behind the xT load).
"""

import sys
import numpy as np

if "/opt/trn_rl_repo" not in sys.path:
    sys.path.insert(0, "/opt/trn_rl_repo")

import ml_dtypes
import concourse.bass as bass
import concourse.bacc as bacc
import concourse.mybir as mybir
import concourse.tile as tile
from concourse.bass_utils import run_bass_kernel_spmd

P = 128
M = 1536          # tokens per expert group
K = 2048          # hidden
N2 = 2816         # 2 * expert_dim (gate | up)
E = 1408          # expert_dim
H = 2048          # hidden (output)

KC = K // P       # 16 contraction chunks, GEMM1
EC = E // P       # 11 contraction chunks, GEMM2 / n-pairs
MT = 512          # m free-dim tile
NMT = M // MT     # 3 m-tiles
MC = M // P       # 12 output m-chunks, GEMM2
HT = 512          # h free-dim tile
NHT = H // HT     # 4 h-tiles

F32 = mybir.dt.float32
F32R = mybir.dt.float32r
BF16 = mybir.dt.bfloat16

IN_DT = BF16      # matmul input dtype (BF16 or F32R)

_cache = {}


def _np_in_dtype():
    return ml_dtypes.bfloat16 if IN_DT == BF16 else np.float32


def _build_nc():
    nc = bacc.Bacc("TRN2", target_bir_lowering=False)

    xT = nc.declare_dram_parameter("xT", [K, M], IN_DT, isOutput=False)
    # w1 arrives host-packed as [pair, gate/up, p, k, c] so each column
    # block is a fully-contiguous DMA (4 KB/partition packets, full rate)
    w1 = nc.declare_dram_parameter("w1", [EC, 2, P, KC, P], IN_DT,
                                   isOutput=False)
    w2 = nc.declare_dram_parameter("w2", [E, H], IN_DT, isOutput=False)
    out = nc.declare_dram_parameter("out", [M, H], F32, isOutput=True)

    with tile.TileContext(nc) as tc:
        with tc.tile_pool(name="act", bufs=1) as act_pool, \
             tc.tile_pool(name="w2p", bufs=1) as w2_pool, \
             tc.tile_pool(name="xt", bufs=1) as xt_pool, \
             tc.tile_pool(name="w1p", bufs=4) as w1_pool, \
             tc.tile_pool(name="silu", bufs=4) as silu_pool, \
             tc.tile_pool(name="ost", bufs=4) as out_pool, \
             tc.tile_pool(name="ps", bufs=8, space="PSUM") as ps_pool:

            # actT: [E, M], resident through both phases
            act_t = [act_pool.tile([P, M], IN_DT, name=f"act{e}", tag=f"act{e}")
                     for e in range(EC)]

            wgs, wus = {}, {}

            def load_pair_weights(i):
                wg = w1_pool.tile([P, KC, P], IN_DT, name=f"wg{i}", tag="wg")
                wu = w1_pool.tile([P, KC, P], IN_DT, name=f"wu{i}", tag="wu")
                nc.sync.dma_start(out=wg, in_=w1[i, 0])
                nc.sync.dma_start(out=wu, in_=w1[i, 1])
                wgs[i], wus[i] = wg, wu

            # ---------------- Phase 1: GEMM1 + SwiGLU ----------------
            # weights stream on the sync queue; pairs 0-2 first
            load_pair_weights(0)
            load_pair_weights(1)
            load_pair_weights(2)

            # xT streams as full [128, M] chunks (3-KB packets), split
            # across both DMA queues: even k on the scalar queue (ahead of
            # nothing), odd k on the sync queue behind pairs 0-2 weights.
            xts = []
            for k in range(KC):
                xt = xt_pool.tile([P, M], IN_DT, name=f"xt{k}", tag=f"xt{k}")
                if k % 2 == 0:
                    nc.scalar.dma_start(out=xt, in_=xT[k * P:(k + 1) * P, :])
                xts.append(xt)
            for k in range(1, KC, 2):
                nc.sync.dma_start(out=xts[k], in_=xT[k * P:(k + 1) * P, :])

            # w_down preload tiles; DMAs are emitted later on the sync queue
            # (after all w1 loads) so they don't compete with xT streaming
            w2ts = [w2_pool.tile([P, H], IN_DT, name=f"w2t{e}", tag=f"w2t{e}")
                    for e in range(EC)]

            def psum_pair(i, t):
                g = ps_pool.tile([P, MT], F32, name=f"psg{i}_{t}", tag="ps")
                u = ps_pool.tile([P, MT], F32, name=f"psu{i}_{t}", tag="ps")
                return g, u

            def swiglu(i, t, g, u):
                tmp = silu_pool.tile([P, MT], F32, name=f"silu{i}_{t}",
                                     tag="silu")
                nc.scalar.activation(
                    out=tmp, in_=g, func=mybir.ActivationFunctionType.Silu)
                nc.vector.tensor_mul(
                    out=act_t[i][:, t * MT:(t + 1) * MT], in0=tmp, in1=u)

            # pairs 0..10 sequential, k-outer/t-inner (6 PSUM banks per
            # pair) — pair 0 chases the xT chunk stream k-sequentially
            for i in range(EC):
                if 2 <= i < EC - 1:
                    load_pair_weights(i + 1)
                elif i == EC - 1:
                    # all w1 queued — append w_down preloads to the sync queue
                    for e in range(EC):
                        nc.sync.dma_start(out=w2ts[e],
                                          in_=w2[e * P:(e + 1) * P, :])
                wg, wu = wgs.pop(i), wus.pop(i)
                pg = [None] * NMT
                pu = [None] * NMT
                for t in range(NMT):
                    pg[t], pu[t] = psum_pair(i, t)
                for k in range(KC):
                    st, sp = (k == 0), (k == KC - 1)
                    for t in range(NMT):
                        xk = xts[k][:, t * MT:(t + 1) * MT]
                        nc.tensor.matmul(pg[t], wg[:, k, :], xk,
                                         start=st, stop=sp)
                        nc.tensor.matmul(pu[t], wu[:, k, :], xk,
                                         start=st, stop=sp)
                for t in range(NMT):
                    swiglu(i, t, pg[t], pu[t])

            # ---------------- Phase 2: GEMM2 ----------------
            for mc in range(MC):
                ps_o = [ps_pool.tile([P, HT], F32, name=f"pso{mc}_{h}",
                                     tag="ps") for h in range(NHT)]
                for e in range(EC):
                    for h in range(NHT):
                        nc.tensor.matmul(
                            ps_o[h],
                            act_t[e][:, mc * P:(mc + 1) * P],
                            w2ts[e][:, h * HT:(h + 1) * HT],
                            start=(e == 0), stop=(e == EC - 1))
                for h in range(NHT):
                    ot = out_pool.tile([P, HT], F32, name=f"ot{mc}_{h}",
                                       tag="ot")
                    nc.vector.tensor_copy(out=ot, in_=ps_o[h])
                    nc.scalar.dma_start(
                        out=out[mc * P:(mc + 1) * P, h * HT:(h + 1) * HT],
                        in_=ot)

    nc.compile()
    return nc


def _pack_w1(w1g):
    # [K, 2*E] -> [pair, gate/up, p, k, c]
    a = w1g.reshape(KC, P, 2, EC, P).transpose(3, 2, 1, 0, 4)
    return np.ascontiguousarray(a)


def make_in_maps(x, w_up_gate, w_down):
    dt = _np_in_dtype()
    in_maps = []
    for g in range(x.shape[0]):
        in_maps.append({
            "xT": np.ascontiguousarray(x[g].T).astype(dt),
            "w1": _pack_w1(w_up_gate[g].astype(dt)),
            "w2": np.ascontiguousarray(w_down[g]).astype(dt),
        })
    return in_maps


def kernel(x, w_up_gate, w_down):
    G = x.shape[0]
    if "nc" not in _cache:
        _cache["nc"] = _build_nc()
    nc = _cache["nc"]
    res = run_bass_kernel_spmd(nc, make_in_maps(x, w_up_gate, w_down),
                               list(range(G)))
    return np.stack([res.results[g]["out"] for g in range(G)], axis=0)


# revision 19
# speedup vs baseline: 1.0536x; 1.0333x over previous
"""DeepSeekMOE grouped-GEMM kernel for 8 Trainium2 NeuronCores.

Expert-parallel: core g handles expert group g.
Per core:  h = x @ w_up_gate ; act = silu(gate)*up ; out = act @ w_down
with x:[1536,2048], w_up_gate:[2048,2816], w_down:[1408,2048] (fp32).

Dataflow (transpose-free on device):
  - host supplies xT = x.T  ([2048,1536]) so both GEMM operands have the
    contraction dim on partitions.
  - GEMM1 computes hT tiles ([n_chunk 128, m 512]) = w1_colblock.T @ xT,
    so SwiGLU output actT lands directly in [E, M] layout — exactly the
    stationary-operand layout GEMM2 needs. out = actT.T @ w_down comes out
    in natural [M, H] orientation.

Matmul inputs are bf16 (host-converted); accumulation stays fp32 in PSUM
and the output is fp32. Set IN_DT = F32R for a TF32-class variant (~2.6e-4
rel err instead of ~5e-3) at ~10% more runtime (doubled DMA traffic).

DMA queues: weights stream on the sync-engine HWDGE queue; xT, w_down and
output tiles on the scalar-engine HWDGE queue (queues execute in emission
order, so the first pair's weights are not stuck behind the xT load).
"""

import sys
import numpy as np

if "/opt/trn_rl_repo" not in sys.path:
    sys.path.insert(0, "/opt/trn_rl_repo")

import ml_dtypes
import concourse.bass as bass
import concourse.bacc as bacc
import concourse.mybir as mybir
import concourse.tile as tile
from concourse.bass_utils import run_bass_kernel_spmd

P = 128
M = 1536          # tokens per expert group
K = 2048          # hidden
N2 = 2816         # 2 * expert_dim (gate | up)
E = 1408          # expert_dim
H = 2048          # hidden (output)

KC = K // P       # 16 contraction chunks, GEMM1
EC = E // P       # 11 contraction chunks, GEMM2 / n-pairs
MT = 512          # m free-dim tile
NMT = M // MT     # 3 m-tiles
MC = M // P       # 12 output m-chunks, GEMM2
HT = 512          # h free-dim tile
NHT = H // HT     # 4 h-tiles

F32 = mybir.dt.float32
F32R = mybir.dt.float32r
BF16 = mybir.dt.bfloat16

IN_DT = BF16      # matmul input dtype (BF16 or F32R)

_cache = {}


def _np_in_dtype():
    return ml_dtypes.bfloat16 if IN_DT == BF16 else np.float32


def _build_nc():
    nc = bacc.Bacc("TRN2", target_bir_lowering=False)

    xT = nc.declare_dram_parameter("xT", [K, M], IN_DT, isOutput=False)
    # w1 arrives host-packed as [pair, gate/up, p, k, c] so each column
    # block is a fully-contiguous DMA (4 KB/partition packets, full rate)
    w1 = nc.declare_dram_parameter("w1", [EC, 2, P, KC, P], IN_DT,
                                   isOutput=False)
    w2 = nc.declare_dram_parameter("w2", [E, H], IN_DT, isOutput=False)
    out = nc.declare_dram_parameter("out", [M, H], F32, isOutput=True)

    with tile.TileContext(nc) as tc:
        with tc.tile_pool(name="act", bufs=1) as act_pool, \
             tc.tile_pool(name="w2p", bufs=1) as w2_pool, \
             tc.tile_pool(name="xt", bufs=1) as xt_pool, \
             tc.tile_pool(name="w1p", bufs=4) as w1_pool, \
             tc.tile_pool(name="silu", bufs=4) as silu_pool, \
             tc.tile_pool(name="ost", bufs=4) as out_pool, \
             tc.tile_pool(name="ps", bufs=8, space="PSUM") as ps_pool:

            # actT: [E, M], resident through both phases
            act_t = [act_pool.tile([P, M], IN_DT, name=f"act{e}", tag=f"act{e}")
                     for e in range(EC)]

            wgs, wus = {}, {}

            def load_pair_weights(i):
                wg = w1_pool.tile([P, KC, P], IN_DT, name=f"wg{i}", tag="wg")
                wu = w1_pool.tile([P, KC, P], IN_DT, name=f"wu{i}", tag="wu")
                nc.sync.dma_start(out=wg, in_=w1[i, 0])
                nc.sync.dma_start(out=wu, in_=w1[i, 1])
                wgs[i], wus[i] = wg, wu

            # ---------------- Phase 1: GEMM1 + SwiGLU ----------------
            # PE warmup: ~36 throwaway matmuls on memset data keep the PE
            # busy through the initial DMA window so the HAM clock-gate is
            # at 2.4 GHz (not 1.2) when the real stream starts.
            warm = act_pool.tile([P, MT], IN_DT, name="warm", tag="warm")
            nc.vector.memset(warm, 0)
            ps_w = ps_pool.tile([P, MT], F32, name="ps_warm", tag="ps")
            WARMUP_N = 36
            for w in range(WARMUP_N):
                nc.tensor.matmul(ps_w, warm[:, :P], warm,
                                 start=(w == 0), stop=(w == WARMUP_N - 1))

            # weights stream on the sync queue; pair 0 first, then the odd
            # xT chunks, then pairs 1-2 (even xT chunks ride the scalar
            # queue from t=0)
            load_pair_weights(0)
            xts = []
            for k in range(KC):
                xt = xt_pool.tile([P, M], IN_DT, name=f"xt{k}", tag=f"xt{k}")
                if k % 2 == 0:
                    nc.scalar.dma_start(out=xt, in_=xT[k * P:(k + 1) * P, :])
                xts.append(xt)
            for k in range(1, KC, 2):
                nc.sync.dma_start(out=xts[k], in_=xT[k * P:(k + 1) * P, :])
            load_pair_weights(1)
            load_pair_weights(2)

            # w_down preload tiles; DMAs are emitted later on the sync queue
            # (after all w1 loads) so they don't compete with xT streaming
            w2ts = [w2_pool.tile([P, H], IN_DT, name=f"w2t{e}", tag=f"w2t{e}")
                    for e in range(EC)]

            def psum_pair(i, t):
                g = ps_pool.tile([P, MT], F32, name=f"psg{i}_{t}", tag="ps")
                u = ps_pool.tile([P, MT], F32, name=f"psu{i}_{t}", tag="ps")
                return g, u

            def swiglu(i, t, g, u):
                tmp = silu_pool.tile([P, MT], F32, name=f"silu{i}_{t}",
                                     tag="silu")
                nc.scalar.activation(
                    out=tmp, in_=g, func=mybir.ActivationFunctionType.Silu)
                nc.vector.tensor_mul(
                    out=act_t[i][:, t * MT:(t + 1) * MT], in0=tmp, in1=u)

            # pairs 0..10 sequential, k-outer/t-inner (6 PSUM banks per
            # pair) — pair 0 chases the xT chunk stream k-sequentially
            for i in range(EC):
                if 2 <= i < EC - 1:
                    load_pair_weights(i + 1)
                elif i == EC - 1:
                    # all w1 queued — append w_down preloads to the sync queue
                    for e in range(EC):
                        nc.sync.dma_start(out=w2ts[e],
                                          in_=w2[e * P:(e + 1) * P, :])
                wg, wu = wgs.pop(i), wus.pop(i)
                pg = [None] * NMT
                pu = [None] * NMT
                for t in range(NMT):
                    pg[t], pu[t] = psum_pair(i, t)
                for k in range(KC):
                    st, sp = (k == 0), (k == KC - 1)
                    for t in range(NMT):
                        xk = xts[k][:, t * MT:(t + 1) * MT]
                        nc.tensor.matmul(pg[t], wg[:, k, :], xk,
                                         start=st, stop=sp)
                        nc.tensor.matmul(pu[t], wu[:, k, :], xk,
                                         start=st, stop=sp)
                for t in range(NMT):
                    swiglu(i, t, pg[t], pu[t])

            # ---------------- Phase 2: GEMM2 ----------------
            for mc in range(MC):
                ps_o = [ps_pool.tile([P, HT], F32, name=f"pso{mc}_{h}",
                                     tag="ps") for h in range(NHT)]
                for e in range(EC):
                    for h in range(NHT):
                        nc.tensor.matmul(
                            ps_o[h],
                            act_t[e][:, mc * P:(mc + 1) * P],
                            w2ts[e][:, h * HT:(h + 1) * HT],
                            start=(e == 0), stop=(e == EC - 1))
                for h in range(NHT):
                    ot = out_pool.tile([P, HT], F32, name=f"ot{mc}_{h}",
                                       tag="ot")
                    nc.vector.tensor_copy(out=ot, in_=ps_o[h])
                    nc.scalar.dma_start(
                        out=out[mc * P:(mc + 1) * P, h * HT:(h + 1) * HT],
                        in_=ot)

    nc.compile()
    return nc


def _pack_w1(w1g):
    # [K, 2*E] -> [pair, gate/up, p, k, c]
    a = w1g.reshape(KC, P, 2, EC, P).transpose(3, 2, 1, 0, 4)
    return np.ascontiguousarray(a)


def make_in_maps(x, w_up_gate, w_down):
    dt = _np_in_dtype()
    in_maps = []
    for g in range(x.shape[0]):
        in_maps.append({
            "xT": np.ascontiguousarray(x[g].T).astype(dt),
            "w1": _pack_w1(w_up_gate[g].astype(dt)),
            "w2": np.ascontiguousarray(w_down[g]).astype(dt),
        })
    return in_maps


def kernel(x, w_up_gate, w_down):
    G = x.shape[0]
    if "nc" not in _cache:
        _cache["nc"] = _build_nc()
    nc = _cache["nc"]
    res = run_bass_kernel_spmd(nc, make_in_maps(x, w_up_gate, w_down),
                               list(range(G)))
    return np.stack([res.results[g]["out"] for g in range(G)], axis=0)


# revision 21
# speedup vs baseline: 1.0633x; 1.0092x over previous
"""DeepSeekMOE grouped-GEMM kernel for 8 Trainium2 NeuronCores.

Expert-parallel: core g handles expert group g.
Per core:  h = x @ w_up_gate ; act = silu(gate)*up ; out = act @ w_down
with x:[1536,2048], w_up_gate:[2048,2816], w_down:[1408,2048] (fp32).

Dataflow (transpose-free on device):
  - host supplies xT = x.T  ([2048,1536]) so both GEMM operands have the
    contraction dim on partitions.
  - GEMM1 computes hT tiles ([n_chunk 128, m 512]) = w1_colblock.T @ xT,
    so SwiGLU output actT lands directly in [E, M] layout — exactly the
    stationary-operand layout GEMM2 needs. out = actT.T @ w_down comes out
    in natural [M, H] orientation.

Matmul inputs are bf16 (host-converted); accumulation stays fp32 in PSUM
and the output is fp32. Set IN_DT = F32R for a TF32-class variant (~2.6e-4
rel err instead of ~5e-3) at ~10% more runtime (doubled DMA traffic).

DMA queues: weights stream on the sync-engine HWDGE queue; xT, w_down and
output tiles on the scalar-engine HWDGE queue (queues execute in emission
order, so the first pair's weights are not stuck behind the xT load).
"""

import sys
import numpy as np

if "/opt/trn_rl_repo" not in sys.path:
    sys.path.insert(0, "/opt/trn_rl_repo")

import ml_dtypes
import concourse.bass as bass
import concourse.bacc as bacc
import concourse.mybir as mybir
import concourse.tile as tile
from concourse.bass_utils import run_bass_kernel_spmd

P = 128
M = 1536          # tokens per expert group
K = 2048          # hidden
N2 = 2816         # 2 * expert_dim (gate | up)
E = 1408          # expert_dim
H = 2048          # hidden (output)

KC = K // P       # 16 contraction chunks, GEMM1
EC = E // P       # 11 contraction chunks, GEMM2 / n-pairs
MT = 512          # m free-dim tile
NMT = M // MT     # 3 m-tiles
MC = M // P       # 12 output m-chunks, GEMM2
HT = 512          # h free-dim tile
NHT = H // HT     # 4 h-tiles

F32 = mybir.dt.float32
F32R = mybir.dt.float32r
BF16 = mybir.dt.bfloat16

IN_DT = BF16      # matmul input dtype (BF16 or F32R)

_cache = {}


def _np_in_dtype():
    return ml_dtypes.bfloat16 if IN_DT == BF16 else np.float32


def _build_nc():
    nc = bacc.Bacc("TRN2", target_bir_lowering=False)

    xT = nc.declare_dram_parameter("xT", [K, M], IN_DT, isOutput=False)
    # w1 arrives host-packed as [pair, gate/up, p, k, c] so each column
    # block is a fully-contiguous DMA (4 KB/partition packets, full rate)
    w1 = nc.declare_dram_parameter("w1", [EC, 2, P, KC, P], IN_DT,
                                   isOutput=False)
    w2 = nc.declare_dram_parameter("w2", [E, H], IN_DT, isOutput=False)
    out = nc.declare_dram_parameter("out", [M, H], F32, isOutput=True)

    with tile.TileContext(nc) as tc:
        with tc.tile_pool(name="act", bufs=1) as act_pool, \
             tc.tile_pool(name="w2p", bufs=1) as w2_pool, \
             tc.tile_pool(name="xt", bufs=1) as xt_pool, \
             tc.tile_pool(name="w1p", bufs=4) as w1_pool, \
             tc.tile_pool(name="silu", bufs=4) as silu_pool, \
             tc.tile_pool(name="ost", bufs=4) as out_pool, \
             tc.tile_pool(name="ps", bufs=8, space="PSUM") as ps_pool:

            # actT: [E, M], resident through both phases
            act_t = [act_pool.tile([P, M], IN_DT, name=f"act{e}", tag=f"act{e}")
                     for e in range(EC)]

            wgs, wus = {}, {}

            def load_pair_weights(i):
                wg = w1_pool.tile([P, KC, P], IN_DT, name=f"wg{i}", tag="wg")
                wu = w1_pool.tile([P, KC, P], IN_DT, name=f"wu{i}", tag="wu")
                nc.sync.dma_start(out=wg, in_=w1[i, 0])
                nc.sync.dma_start(out=wu, in_=w1[i, 1])
                wgs[i], wus[i] = wg, wu

            # ---------------- Phase 1: GEMM1 + SwiGLU ----------------
            # PE warmup: ~36 throwaway matmuls on memset data keep the PE
            # busy through the initial DMA window so the HAM clock-gate is
            # at 2.4 GHz (not 1.2) when the real stream starts.
            warm = act_pool.tile([P, MT], IN_DT, name="warm", tag="warm")
            nc.vector.memset(warm, 0)
            ps_w = ps_pool.tile([P, MT], F32, name="ps_warm", tag="ps")
            WARMUP_N = 28
            for w in range(WARMUP_N):
                nc.tensor.matmul(ps_w, warm[:, :P], warm,
                                 start=(w == 0), stop=(w == WARMUP_N - 1))

            # weights stream on the sync queue; pair 0 first, then the odd
            # xT chunks, then pairs 1-2 (even xT chunks ride the scalar
            # queue from t=0)
            load_pair_weights(0)
            xts = []
            for k in range(KC):
                xt = xt_pool.tile([P, M], IN_DT, name=f"xt{k}", tag=f"xt{k}")
                if k % 2 == 0:
                    nc.scalar.dma_start(out=xt, in_=xT[k * P:(k + 1) * P, :])
                xts.append(xt)
            for k in range(1, KC, 2):
                nc.sync.dma_start(out=xts[k], in_=xT[k * P:(k + 1) * P, :])
            load_pair_weights(1)
            load_pair_weights(2)

            # w_down preload tiles; DMAs are emitted later on the sync queue
            # (after all w1 loads) so they don't compete with xT streaming
            w2ts = [w2_pool.tile([P, H], IN_DT, name=f"w2t{e}", tag=f"w2t{e}")
                    for e in range(EC)]

            def psum_pair(i, t):
                g = ps_pool.tile([P, MT], F32, name=f"psg{i}_{t}", tag="ps")
                u = ps_pool.tile([P, MT], F32, name=f"psu{i}_{t}", tag="ps")
                return g, u

            def swiglu(i, t, g, u):
                tmp = silu_pool.tile([P, MT], F32, name=f"silu{i}_{t}",
                                     tag="silu")
                nc.scalar.activation(
                    out=tmp, in_=g, func=mybir.ActivationFunctionType.Silu)
                nc.vector.tensor_mul(
                    out=act_t[i][:, t * MT:(t + 1) * MT], in0=tmp, in1=u)

            # pairs 0..10 sequential, k-outer/t-inner (6 PSUM banks per
            # pair) — pair 0 chases the xT chunk stream k-sequentially
            for i in range(EC):
                if 2 <= i < EC - 1:
                    load_pair_weights(i + 1)
                elif i == EC - 1:
                    # all w1 queued — append w_down preloads to the sync queue
                    for e in range(EC):
                        nc.sync.dma_start(out=w2ts[e],
                                          in_=w2[e * P:(e + 1) * P, :])
                wg, wu = wgs.pop(i), wus.pop(i)
                pg = [None] * NMT
                pu = [None] * NMT
                for t in range(NMT):
                    pg[t], pu[t] = psum_pair(i, t)
                for k in range(KC):
                    st, sp = (k == 0), (k == KC - 1)
                    for t in range(NMT):
                        xk = xts[k][:, t * MT:(t + 1) * MT]
                        nc.tensor.matmul(pg[t], wg[:, k, :], xk,
                                         start=st, stop=sp)
                        nc.tensor.matmul(pu[t], wu[:, k, :], xk,
                                         start=st, stop=sp)
                for t in range(NMT):
                    swiglu(i, t, pg[t], pu[t])

            # ---------------- Phase 2: GEMM2 ----------------
            # h-outer/e-inner: each output bank finishes after 11 MMs so
            # its copy + store overlap the next bank's accumulation
            for mc in range(MC):
                for h in range(NHT):
                    ps_o = ps_pool.tile([P, HT], F32, name=f"pso{mc}_{h}",
                                        tag="ps")
                    for e in range(EC):
                        nc.tensor.matmul(
                            ps_o,
                            act_t[e][:, mc * P:(mc + 1) * P],
                            w2ts[e][:, h * HT:(h + 1) * HT],
                            start=(e == 0), stop=(e == EC - 1))
                    ot = out_pool.tile([P, HT], F32, name=f"ot{mc}_{h}",
                                       tag="ot")
                    nc.vector.tensor_copy(out=ot, in_=ps_o)
                    nc.scalar.dma_start(
                        out=out[mc * P:(mc + 1) * P, h * HT:(h + 1) * HT],
                        in_=ot)

    nc.compile()
    return nc


def _pack_w1(w1g):
    # [K, 2*E] -> [pair, gate/up, p, k, c]
    a = w1g.reshape(KC, P, 2, EC, P).transpose(3, 2, 1, 0, 4)
    return np.ascontiguousarray(a)


def make_in_maps(x, w_up_gate, w_down):
    dt = _np_in_dtype()
    in_maps = []
    for g in range(x.shape[0]):
        in_maps.append({
            "xT": np.ascontiguousarray(x[g].T).astype(dt),
            "w1": _pack_w1(w_up_gate[g].astype(dt)),
            "w2": np.ascontiguousarray(w_down[g]).astype(dt),
        })
    return in_maps


def kernel(x, w_up_gate, w_down):
    G = x.shape[0]
    if "nc" not in _cache:
        _cache["nc"] = _build_nc()
    nc = _cache["nc"]
    res = run_bass_kernel_spmd(nc, make_in_maps(x, w_up_gate, w_down),
                               list(range(G)))
    return np.stack([res.results[g]["out"] for g in range(G)], axis=0)
